# revision 26
# baseline (speedup 1.0000x reference)
"""Trainium2 Bass kernel: end-to-end model (pool -> linear -> max/argmax ->
top-k -> gather) distributed over 8 NeuronCores.

v3: one saturated DMA stream with everything pipelined underneath it.
  - x (uint8, no padding, 9.83MB/core) streams first; pooling (DVE samples
    0-3, ACT samples 4-7) consumes tiles as they land.
  - W is column-sharded (38 of 304 padded queries per core) and streamed
    CHUNK-major (query-chunk granules of [128|48, cols]) right behind x;
    the matmul accumulates each chunk over the 10 k-tiles as granules
    arrive, so chunk g's postproc runs while chunk g+1 still streams.
  - Pooled features are AllGathered in 4 tile-groups (hidden under the W
    stream); per-query results AllToAll'd in two pieces (chunks 0-5, then
    chunk 6 + scores) so the top-k tail starts as early as possible.
  - Tail: 19 rounds of max8/max_index/match_replace + gpsimd ap_gather.

Contraction is over the real 1200 rows only: 9 full 128-row k-tiles plus
one 48-row partial tile (no zero padding in x or W).

Self-contained: hardcodes all shapes; builds one SPMD Bass program and runs
it via run_bass_kernel_spmd on cores 0-7.
"""

import os
import sys
from contextlib import ExitStack

import numpy as np

for _p in ("/opt/trn_rl_repo", "/root/.axon_site/_ro/trn_rl_repo"):
    if os.path.isdir(_p) and _p not in sys.path:
        sys.path.append(_p)

import concourse.bass as bass
import concourse.tile as tile
from concourse import bacc, library_config, mybir
from concourse.bass_utils import run_bass_kernel_spmd

dt = mybir.dt
F32 = dt.float32
AX = mybir.AxisListType
OP = mybir.AluOpType

# ---------------- problem constants (hardcoded) ----------------
B, CHN, HIMG, WIMG = 64, 3, 640, 640
NQ, NCHAN, NCL, TOPK = 300, 84, 80, 150
KDIM, NOUT = 1200, 25200           # 3*20*20, NQ*NCHAN
NCORES = 8
BPC = B // NCORES                  # samples per core = 8
SCALE = np.float64(1.0) / (32 * 32 * 255)
NEG = -3.0e38
NIDX = 160                         # padded top-k index count (152 used)
NROUND = 19                        # 19 rounds x 8 = 152 >= 150

NQP = 304                          # padded query count (8 * 38)
QN = NQP // NCORES                 # queries per core = 38
NT = 10                            # k-tiles: 9 full (128 rows) + 1 partial
TFULL = 9
PPART = KDIM - 128 * TFULL         # rows in partial tile = 48
CHQ = [6, 6, 6, 6, 6, 6, 2]        # queries per chunk (psum bank = 512 f32)
COLS = [q * NCHAN for q in CHQ]    # [504]*6 + [168]
NCHUNKS = len(CHQ)
AGG = [(0, 2), (2, 5), (5, 8), (8, 10)]  # AllGather tile groups
RG = [list(range(NCORES))]         # one replica group: all 8 cores

# matmul dtype: f32 (safe) or f32r (4x PE, near-f32 precision; verify on HW)
MM_DTYPE = F32 if os.environ.get("MM_F32R", "0") != "1" else dt.float32r

GRAN = [(0, 48), (48, 96), (96, 144), (144, 160)]  # gather granules


def build_program():
    nc = bacc.Bacc("TRN2", target_bir_lowering=False, debug=False,
                   num_devices=NCORES)
    # x host-packed cell-major, unpadded: tile t<9 holds cells 128t..128t+127
    # on partitions, free (b, pix); partial tile holds cells 1152..1199 on
    # partitions 0..47.
    xf_d = nc.dram_tensor("xf", [128, TFULL * BPC * 1024], dt.uint8,
                          kind="ExternalInput")
    xp_d = nc.dram_tensor("xp", [PPART, BPC * 1024], dt.uint8,
                          kind="ExternalInput")
    # W shard host-packed chunk-major: [chunk, tile, 128|48 rows, cols]
    wA_d = nc.dram_tensor("wA", [6, TFULL, 128, COLS[0]], MM_DTYPE,
                          kind="ExternalInput")
    wAp_d = nc.dram_tensor("wAp", [6, PPART, COLS[0]], MM_DTYPE,
                           kind="ExternalInput")
    wB_d = nc.dram_tensor("wB", [TFULL, 128, COLS[6]], MM_DTYPE,
                          kind="ExternalInput")
    wBp_d = nc.dram_tensor("wBp", [PPART, COLS[6]], MM_DTYPE,
                           kind="ExternalInput")
    iod_d = nc.dram_tensor("iod", [B, NCL], F32, kind="ExternalInput")
    out_d = nc.dram_tensor("out", [BPC, TOPK, 6], F32, kind="ExternalOutput")

    with tile.TileContext(nc) as tc:
        with ExitStack() as ctx:
            _body(ctx, tc, xf_d, xp_d, (wA_d, wAp_d, wB_d, wBp_d), iod_d,
                  out_d)
    nc.finalize()
    return nc


def _body(ctx, tc, xf_d, xp_d, w_d, iod_d, out_d):
    nc = tc.nc
    wA_d, wAp_d, wB_d, wBp_d = w_d

    # ---------------- persistent tiles ----------------
    P = ctx.enter_context(tc.tile_pool(name="persist", bufs=1))

    iod = P.tile([B, NCL], F32, tag="iod")
    s_pool = P.tile([128, NT * BPC], F32, tag="s_pool")   # [p, (t, b)]
    acts = P.tile([128, 1024], F32, tag="acts")           # ACT accum dump
    pt = P.tile([128, NT * B], MM_DTYPE, tag="pt")        # [p, (t, sample)]

    scores = P.tile([B, QN], F32, tag="scores")
    eq = P.tile([B, CHQ[0] * NCL], F32, tag="eq")
    am = P.tile([B, CHQ[0] * NCL], F32, tag="am")
    argt = P.tile([B, CHQ[0]], F32, tag="argt")
    a2a_sb = P.tile([B, QN * 6 + QN], F32, tag="a2a_sb")  # rows + scores

    feat = P.tile([128, NQP * 6], F32, tag="feat")        # gather source
    swk = P.tile([BPC, NQP], F32, tag="swk")              # topk scratch
    tv = P.tile([BPC, NROUND * 8], F32, tag="tv")
    ti = P.tile([BPC, NROUND * 8], dt.uint32, tag="ti")
    ti16 = P.tile([BPC, NIDX], dt.int16, tag="ti16")
    wraps = [P.tile([128, (i1 - i0) // 16], dt.int16, tag=f"wrap{h}",
                    name=f"wrap{h}")
             for h, (i0, i1) in enumerate(GRAN)]
    gout = P.tile([128, NIDX * 6], F32, tag="gout")

    # DRAM bounce buffers for the collectives
    DP = ctx.enter_context(tc.tile_pool(name="dram", bufs=1, space="DRAM"))
    ag_in = [DP.tile([128, (t1 - t0) * BPC], MM_DTYPE, tag=f"agi{g}",
                     name=f"agi{g}")
             for g, (t0, t1) in enumerate(AGG)]
    ag_out = [DP.tile([NCORES, 128, (t1 - t0) * BPC], MM_DTYPE, tag=f"ago{g}",
                      name=f"ago{g}")
              for g, (t0, t1) in enumerate(AGG)]
    a2a_in = DP.tile([B, QN * 6 + QN], F32, tag="a2a_in")
    a2a_out = DP.tile([NCORES, BPC, QN * 6 + QN], F32, tag="a2a_out")
    tsc = DP.tile([BPC, NIDX], dt.int16, tag="tsc")
    wu_in = DP.tile([1, 8], F32, tag="wu_in")
    wu_out = DP.tile([NCORES, 1, 8], F32, tag="wu_out")

    nc.gpsimd.memset(ti16[:, :], 0)
    nc.gpsimd.memset(feat[:, :], 0)
    # partial-tile rows 48.. of s_pool are never written by pooling; zero the
    # whole tile so the AllGather of group 3 doesn't ship uninitialized SBUF.
    nc.vector.memset(s_pool[:, :], 0.0)

    spv = s_pool[:].rearrange("p (t b) -> p t b", t=NT)

    # ---------------- x stream + pooling, self-throttled -------------------
    # x tiles rotate through a 3-deep pool: the sync queue can only run a
    # couple of tiles ahead of pooling, so the tiny AllGather bounces and
    # the W stream interleave into the DMA pipeline early instead of
    # queueing behind 10MB of x.
    # DVE reduces samples 0-4 of each tile in one op; ACT accumulates
    # samples 5-7. AG shuffles ride the tensor queue: each blocks exactly
    # the matmuls that need it.
    nc.sync.dma_start(iod[:], iod_d[:])
    XP = ctx.enter_context(tc.tile_pool(name="xp", bufs=5))
    with nc.allow_low_precision(reason="f32 sums of uint8 are exact"):
        for t in range(NT):
            np_ = 128 if t < TFULL else PPART
            xt = XP.tile([128, BPC * 1024], dt.uint8, tag="x", name=f"x{t}")
            if t < TFULL:
                nc.sync.dma_start(
                    xt[:], xf_d[:, t * BPC * 1024:(t + 1) * BPC * 1024])
            else:
                nc.sync.dma_start(xt[:np_, :], xp_d[:])
            xv = xt[:np_, :].rearrange("p (b x) -> p b x", x=1024)
            nc.vector.tensor_reduce(
                spv[:np_, t, 0:4], xv[:, 0:4, :], axis=AX.X, op=OP.add
            )
            for b in range(4, BPC):
                nc.scalar.activation(
                    acts[:np_, :],
                    xt[:np_, b * 1024:(b + 1) * 1024],
                    mybir.ActivationFunctionType.Copy,
                    accum_out=spv[:np_, t, b:b + 1],
                )
            # grouped AllGather once the last tile of a group is pooled;
            # the pt shuffle rides the scalar queue right after the group's
            # ACT accums so it blocks nothing that is needed earlier.
            for g, (t0, t1) in enumerate(AGG):
                if t == t1 - 1:
                    nc.gpsimd.dma_start(ag_in[g][:], spv[:, t0:t1, :])
                    nc.gpsimd.collective_compute(
                        "AllGather", OP.bypass, replica_groups=RG,
                        ins=[ag_in[g].opt()], outs=[ag_out[g].opt()],
                    )
                    nc.gpsimd.dma_start(
                        pt[:].rearrange("p (t c b) -> p t c b", t=NT, c=NCORES)[:, t0:t1],
                        ag_out[g][:].rearrange(
                            "c p (t b) -> p t c b", t=t1 - t0),
                    )

    # ---------------- matmul (chunk-major) + per-chunk postproc ------------
    ptv = pt[:].rearrange("p (t s) -> p t s", t=NT)
    a2v = a2a_sb[:, :QN * 6].rearrange("b (q c) -> b q c", c=6)
    sc_all = a2a_sb[:, QN * 6:]    # [B, 38] score row

    WP = ctx.enter_context(tc.tile_pool(name="wp", bufs=60))
    WPB = ctx.enter_context(tc.tile_pool(name="wpb", bufs=10))
    YPS = ctx.enter_context(tc.tile_pool(name="yps", bufs=7, space="PSUM"))

    def postproc(g, psy):
        nq = CHQ[g]
        cols = nq * NCHAN
        psv = psy[:, :cols].rearrange("b (q c) -> b q c", c=NCHAN)
        q0 = 6 * g
        sg = scores[:, g * 6: g * 6 + nq]
        # boxes + score copies ride the (otherwise idle) ACT engine
        nc.scalar.activation(
            a2v[:, q0:q0 + nq, 0:4], psv[:, :, 0:4],
            mybir.ActivationFunctionType.Copy,
        )
        nc.vector.tensor_reduce(sg, psv[:, :, 4:NCHAN], axis=AX.X, op=OP.max)
        eqv = eq[:, :nq * NCL].rearrange("b (q c) -> b q c", c=NCL)
        nc.vector.tensor_tensor(
            eqv, psv[:, :, 4:NCHAN],
            sg.unsqueeze(-1).broadcast_to((B, nq, NCL)),
            op=OP.is_ge,
        )
        amv = am[:, :nq * NCL].rearrange("b (q c) -> b q c", c=NCL)
        nc.vector.tensor_tensor(
            amv, eqv, iod[:, :].unsqueeze(1).broadcast_to((B, nq, NCL)),
            op=OP.mult,
        )
        nc.vector.tensor_reduce(argt[:, :nq], amv, axis=AX.X, op=OP.max)
        nc.vector.tensor_scalar(
            a2v[:, q0:q0 + nq, 5], argt[:, :nq], -1.0, float(NCL - 1),
            op0=OP.mult, op1=OP.add,
        )
        nc.scalar.activation(
            sc_all[:, g * 6: g * 6 + nq], sg,
            mybir.ActivationFunctionType.Copy,
        )
        nc.vector.tensor_copy(a2v[:, q0:q0 + nq, 4], sg)

    # gpsimd gather library: loading it reflashes the Q7 complex and keeps
    # the CC cores unavailable for ~70us, so it must come AFTER the
    # AllGather ladder; its drain overlaps the matmul phase and finishes
    # long before the tail's ap_gathers.
    nc.gpsimd.load_library(library_config.ap_gather)

    def w_src(g, t):
        if g < 6:
            return wA_d[g, t] if t < TFULL else wAp_d[g]
        return wB_d[t] if t < TFULL else wBp_d[:]

    psys = [YPS.tile([B, COLS[0]], F32, tag="psy", name=f"psy{g}")
            for g in range(NCHUNKS)]

    def mm(g, t, stop):
        cols = COLS[g]
        np_ = 128 if t < TFULL else PPART
        if g < 6:
            wt = WP.tile([128, COLS[0]], MM_DTYPE, tag="wt", name=f"wt{g}_{t}")
        else:
            wt = WPB.tile([128, COLS[6]], MM_DTYPE, tag="wtb", name=f"wt{g}_{t}")
        nc.sync.dma_start(wt[:np_, :cols], w_src(g, t))
        nc.tensor.matmul(
            psys[g][:, :cols], ptv[:np_, t, :], wt[:np_, :cols],
            start=(t == 0), stop=stop,
        )

    # t-major: PE runs gapless behind the W stream, gated per t-group on
    # the (strictly serialized) AllGathers; postproc pipelines per chunk
    # after its t9 matmul.
    for t in range(NT - 1):
        for g in range(NCHUNKS):
            mm(g, t, stop=False)
    for g in range(NCHUNKS):
        mm(g, NT - 1, stop=True)
        postproc(g, psys[g])
        if g < 6:
            nc.scalar.dma_start(
                a2a_in[:, 36 * g: 36 * (g + 1)],
                a2a_sb[:, 36 * g: 36 * (g + 1)])
        else:
            nc.scalar.dma_start(a2a_in[:, 216:], a2a_sb[:, 216:])

    nc.gpsimd.collective_compute(
        "AllToAll", OP.bypass, replica_groups=RG,
        ins=[a2a_in.opt()], outs=[a2a_out.opt()],
    )
    # feat[16b] rows: sample b's [304, 6] = concat of 8 cores' blocks
    nc.gpsimd.dma_start(
        swk[:].rearrange("b (c q) -> b c q", c=NCORES),
        a2a_out[:, :, QN * 6:].rearrange("c b q -> b c q"),
    )
    nc.gpsimd.dma_start(
        feat[:].rearrange("(b s) (c q x) -> b s c q x",
                          b=BPC, c=NCORES, x=6)[:, 0],
        a2a_out[:, :, :QN * 6].rearrange("c b (q x) -> b c q x", x=6),
    )
    nc.vector.memset(swk[:, NQ:NQP], NEG)  # padded queries never win

    # ---------------- top-150 tail -----------------------------------------
    # indices from early rounds are wrapped + gathered while later rounds
    # still run on DVE
    def wrap_and_gather(h):
        i0, i1 = GRAN[h]
        ic = min(i1, NROUND * 8)
        nc.vector.tensor_copy(ti16[:, i0:ic], ti[:, i0:ic])
        nc.scalar.dma_start(tsc[:, i0:i1], ti16[:, i0:i1])
        for b in range(BPC):
            eng = nc.sync if b % 2 == 0 else nc.scalar
            eng.dma_start(
                wraps[h][16 * b: 16 * b + 16, :],
                tsc[b, i0:i1].rearrange("(f p) -> p f", p=16),
            )
        nc.gpsimd.ap_gather(
            gout[:].rearrange("p (i c) -> p i c", c=6)[:, i0:i1],
            feat[:].rearrange("p (q c) -> p q c", c=6),
            wraps[h][:],
            channels=128,
            num_elems=NQP,
            d=6,
            num_idxs=i1 - i0,
        )
        o1 = min(i1, TOPK)
        if i0 < TOPK:
            nc.sync.dma_start(
                out_d[:].rearrange("b k c -> b (k c)")[:, i0 * 6: o1 * 6],
                gout[:].rearrange("(b s) x -> b s x", b=BPC)[
                    :, 0, i0 * 6: o1 * 6],
            )

    for r in range(NROUND):
        nc.vector.max(tv[:, 8 * r: 8 * r + 8], swk[:, :])
        nc.vector.max_index(ti[:, 8 * r: 8 * r + 8], tv[:, 8 * r: 8 * r + 8],
                            swk[:, :])
        if r < NROUND - 1:
            nc.vector.match_replace(
                swk[:, :], tv[:, 8 * r: 8 * r + 8], swk[:, :], NEG
            )
        if r == 5:
            wrap_and_gather(0)
        elif r == 11:
            wrap_and_gather(1)
        elif r == 17:
            wrap_and_gather(2)
    wrap_and_gather(3)




def _make_consts():
    return np.broadcast_to(
        (np.float32(NCL - 1) - np.arange(NCL, dtype=np.float32))[None, :],
        (B, NCL),
    ).copy()


_NC_CACHE = {}


def _get_nc():
    if "nc" not in _NC_CACHE:
        _NC_CACHE["nc"] = build_program()
    return _NC_CACHE["nc"]


def pack_x(xs: np.ndarray) -> tuple[np.ndarray, np.ndarray]:
    """[BPC, 3, 640, 640] int32 -> unpadded cell-major uint8 tiles.

    Cell k = c_rgb*400 + i*20 + j (matching W's row layout after the
    BGR->RGB flip); tile t<9: cell 128t+p at partition p, free b*1024+pix;
    partial tile: cells 1152..1199 on partitions 0..47.
    """
    xs8 = xs.astype(np.uint8).reshape(BPC, CHN, 20, 32, 20, 32)
    xs8 = xs8[:, ::-1]  # BGR -> RGB
    cells = xs8.transpose(0, 1, 2, 4, 3, 5).reshape(BPC, KDIM, 1024)
    full = cells[:, :128 * TFULL].reshape(BPC, TFULL, 128, 1024)
    xf = np.ascontiguousarray(full.transpose(2, 1, 0, 3)).reshape(
        128, TFULL * BPC * 1024)
    xp = np.ascontiguousarray(
        cells[:, 128 * TFULL:].transpose(1, 0, 2)).reshape(PPART, BPC * 1024)
    return xf, xp


def pack_w(W: np.ndarray):
    """[1200, 25200] -> per-core chunk-major granule tensors (scale folded).

    Returns (wA [8,6,9,128,504], wAp [8,6,48,504], wB [8,9,128,168],
    wBp [8,48,168]).
    """
    Wp = np.zeros((KDIM, NQP * NCHAN), np.float32)
    Wp[:, : NQ * NCHAN] = (W.astype(np.float64) * SCALE).astype(np.float32)
    wA = np.zeros((NCORES, 6, TFULL, 128, COLS[0]), np.float32)
    wAp = np.zeros((NCORES, 6, PPART, COLS[0]), np.float32)
    wB = np.zeros((NCORES, TFULL, 128, COLS[6]), np.float32)
    wBp = np.zeros((NCORES, PPART, COLS[6]), np.float32)
    for c in range(NCORES):
        s = Wp[:, c * QN * NCHAN: (c + 1) * QN * NCHAN]
        q0 = 0
        for g in range(NCHUNKS):
            cols = COLS[g]
            blk = s[:, q0: q0 + cols]
            if g < 6:
                wA[c, g] = blk[:128 * TFULL].reshape(TFULL, 128, cols)
                wAp[c, g] = blk[128 * TFULL:]
            else:
                wB[c] = blk[:128 * TFULL].reshape(TFULL, 128, cols)
                wBp[c] = blk[128 * TFULL:]
            q0 += cols
    return wA, wAp, wB, wBp


def make_in_maps(x: np.ndarray, W: np.ndarray) -> list[dict]:
    iod = _make_consts()
    wA, wAp, wB, wBp = pack_w(W)
    in_maps = []
    for c in range(NCORES):
        xf, xp = pack_x(x[c * BPC: (c + 1) * BPC])
        in_maps.append(
            {
                "xf": xf,
                "xp": xp,
                "wA": wA[c],
                "wAp": wAp[c],
                "wB": wB[c],
                "wBp": wBp[c],
                "iod": iod,
            }
        )
    return in_maps


def kernel(x: np.ndarray, W: np.ndarray) -> np.ndarray:
    x = np.ascontiguousarray(np.asarray(x), dtype=np.int32)
    W = np.ascontiguousarray(np.asarray(W), dtype=np.float32)
    assert x.shape == (B, CHN, HIMG, WIMG) and W.shape == (KDIM, NOUT)

    nc = _get_nc()
    in_maps = make_in_maps(x, W)
    res = run_bass_kernel_spmd(nc, in_maps, core_ids=list(range(NCORES)))
    out = np.concatenate([res.results[c]["out"] for c in range(NCORES)], axis=0)
    return out.astype(np.float32)


if __name__ == "__main__":
    xs = np.random.randint(0, 256, (B, CHN, HIMG, WIMG)).astype(np.int32)
    Ws = (np.random.randn(KDIM, NOUT) * 0.02).astype(np.float32)
    o = kernel(xs, Ws)
    print("kernel output:", o.shape, o.dtype)


# revision 27
# speedup vs baseline: 1.4215x; 1.4215x over previous
"""Trainium2 Bass kernel: end-to-end model (pool -> linear -> max/argmax ->
top-k -> gather) distributed over 8 NeuronCores.

v3: one saturated DMA stream with everything pipelined underneath it.
  - x (uint8, no padding, 9.83MB/core) streams first; pooling (DVE samples
    0-3, ACT samples 4-7) consumes tiles as they land.
  - W is column-sharded (38 of 304 padded queries per core) and streamed
    CHUNK-major (query-chunk granules of [128|48, cols]) right behind x;
    the matmul accumulates each chunk over the 10 k-tiles as granules
    arrive, so chunk g's postproc runs while chunk g+1 still streams.
  - Pooled features are AllGathered in 4 tile-groups (hidden under the W
    stream); per-query results AllToAll'd in two pieces (chunks 0-5, then
    chunk 6 + scores) so the top-k tail starts as early as possible.
  - Tail: 19 rounds of max8/max_index/match_replace + gpsimd ap_gather.

Contraction is over the real 1200 rows only: 9 full 128-row k-tiles plus
one 48-row partial tile (no zero padding in x or W).

Self-contained: hardcodes all shapes; builds one SPMD Bass program and runs
it via run_bass_kernel_spmd on cores 0-7.
"""

import os
import sys
from contextlib import ExitStack

import numpy as np

for _p in ("/opt/trn_rl_repo", "/root/.axon_site/_ro/trn_rl_repo"):
    if os.path.isdir(_p) and _p not in sys.path:
        sys.path.append(_p)

import concourse.bass as bass
import concourse.tile as tile
from concourse import bacc, library_config, mybir
from concourse.bass_utils import run_bass_kernel_spmd

dt = mybir.dt
F32 = dt.float32
AX = mybir.AxisListType
OP = mybir.AluOpType

# ---------------- problem constants (hardcoded) ----------------
B, CHN, HIMG, WIMG = 64, 3, 640, 640
NQ, NCHAN, NCL, TOPK = 300, 84, 80, 150
KDIM, NOUT = 1200, 25200           # 3*20*20, NQ*NCHAN
NCORES = 8
BPC = B // NCORES                  # samples per core = 8
SCALE = np.float64(1.0) / (32 * 32 * 255)
NEG = -3.0e38
NIDX = 160                         # padded top-k index count (152 used)
NROUND = 19                        # 19 rounds x 8 = 152 >= 150

NQP = 304                          # padded query count (8 * 38)
QN = NQP // NCORES                 # queries per core = 38
NT = 10                            # k-tiles: 9 full (128 rows) + 1 partial
TFULL = 9
PPART = KDIM - 128 * TFULL         # rows in partial tile = 48
CHQ = [6, 6, 6, 6, 6, 6, 2]        # queries per chunk (psum bank = 512 f32)
COLS = [q * NCHAN for q in CHQ]    # [504]*6 + [168]
NCHUNKS = len(CHQ)
AGG = [(0, 2), (2, 5), (5, 8), (8, 10)]  # AllGather tile groups
RG = [list(range(NCORES))]         # one replica group: all 8 cores

# matmul dtype: f32 (safe) or f32r (4x PE, near-f32 precision; verify on HW)
MM_DTYPE = F32 if os.environ.get("MM_F32R", "0") != "1" else dt.float32r

GRAN = [(0, 48), (48, 96), (96, 144), (144, 160)]  # gather granules


def build_program():
    nc = bacc.Bacc("TRN2", target_bir_lowering=False, debug=False,
                   num_devices=NCORES)
    # x host-packed cell-major, unpadded: tile t<9 holds cells 128t..128t+127
    # on partitions, free (b, pix); partial tile holds cells 1152..1199 on
    # partitions 0..47.
    xf_d = nc.dram_tensor("xf", [128, TFULL * BPC * 1024], dt.uint8,
                          kind="ExternalInput")
    xp_d = nc.dram_tensor("xp", [PPART, BPC * 1024], dt.uint8,
                          kind="ExternalInput")
    # W shard host-packed chunk-major: [chunk, tile, 128|48 rows, cols]
    wA_d = nc.dram_tensor("wA", [6, TFULL, 128, COLS[0]], MM_DTYPE,
                          kind="ExternalInput")
    wAp_d = nc.dram_tensor("wAp", [6, PPART, COLS[0]], MM_DTYPE,
                           kind="ExternalInput")
    wB_d = nc.dram_tensor("wB", [TFULL, 128, COLS[6]], MM_DTYPE,
                          kind="ExternalInput")
    wBp_d = nc.dram_tensor("wBp", [PPART, COLS[6]], MM_DTYPE,
                           kind="ExternalInput")
    iod_d = nc.dram_tensor("iod", [B, NCL], F32, kind="ExternalInput")
    out_d = nc.dram_tensor("out", [BPC, TOPK, 6], F32, kind="ExternalOutput")

    with tile.TileContext(nc) as tc:
        with ExitStack() as ctx:
            _body(ctx, tc, xf_d, xp_d, (wA_d, wAp_d, wB_d, wBp_d), iod_d,
                  out_d)
    nc.finalize()
    return nc


def _body(ctx, tc, xf_d, xp_d, w_d, iod_d, out_d):
    nc = tc.nc
    wA_d, wAp_d, wB_d, wBp_d = w_d

    # ---------------- persistent tiles ----------------
    P = ctx.enter_context(tc.tile_pool(name="persist", bufs=1))

    iod = P.tile([B, NCL], F32, tag="iod")
    s_pool = P.tile([128, NT * BPC], F32, tag="s_pool")   # [p, (t, b)]
    acts = P.tile([128, 1024], F32, tag="acts")           # ACT accum dump
    pt = P.tile([128, NT * B], MM_DTYPE, tag="pt")        # [p, (t, sample)]

    scores = P.tile([B, QN], F32, tag="scores")
    eq = P.tile([B, CHQ[0] * NCL], F32, tag="eq")
    am = P.tile([B, CHQ[0] * NCL], F32, tag="am")
    argt = P.tile([B, CHQ[0]], F32, tag="argt")
    a2a_sb = P.tile([B, QN * 6 + QN], F32, tag="a2a_sb")  # rows + scores

    feat = P.tile([128, NQP * 6], F32, tag="feat")        # gather source
    swk = P.tile([BPC, NQP], F32, tag="swk")              # topk scratch
    tv = P.tile([BPC, NROUND * 8], F32, tag="tv")
    ti = P.tile([BPC, NROUND * 8], dt.uint32, tag="ti")
    ti16 = P.tile([BPC, NIDX], dt.int16, tag="ti16")
    wraps = [P.tile([128, (i1 - i0) // 16], dt.int16, tag=f"wrap{h}",
                    name=f"wrap{h}")
             for h, (i0, i1) in enumerate(GRAN)]
    gout = P.tile([128, NIDX * 6], F32, tag="gout")

    # DRAM bounce buffers for the collectives
    DP = ctx.enter_context(tc.tile_pool(name="dram", bufs=1, space="DRAM"))
    ag_in = [DP.tile([128, (t1 - t0) * BPC], MM_DTYPE, tag=f"agi{g}",
                     name=f"agi{g}")
             for g, (t0, t1) in enumerate(AGG)]
    ag_out = [DP.tile([NCORES, 128, (t1 - t0) * BPC], MM_DTYPE, tag=f"ago{g}",
                      name=f"ago{g}")
              for g, (t0, t1) in enumerate(AGG)]
    a2a_in = DP.tile([B, QN * 6 + QN], F32, tag="a2a_in")
    a2a_out = DP.tile([NCORES, BPC, QN * 6 + QN], F32, tag="a2a_out")
    tsc = DP.tile([BPC, NIDX], dt.int16, tag="tsc")
    wu_in = DP.tile([1, 8], F32, tag="wu_in")
    wu_out = DP.tile([NCORES, 1, 8], F32, tag="wu_out")

    nc.gpsimd.memset(ti16[:, :], 0)
    nc.gpsimd.memset(feat[:, :], 0)
    # partial-tile rows 48.. of s_pool are never written by pooling; zero the
    # whole tile so the AllGather of group 3 doesn't ship uninitialized SBUF.
    nc.vector.memset(s_pool[:, :], 0.0)

    spv = s_pool[:].rearrange("p (t b) -> p t b", t=NT)

    # ---------------- x stream + pooling, self-throttled -------------------
    # x tiles rotate through a 3-deep pool: the sync queue can only run a
    # couple of tiles ahead of pooling, so the tiny AllGather bounces and
    # the W stream interleave into the DMA pipeline early instead of
    # queueing behind 10MB of x.
    # DVE reduces samples 0-4 of each tile in one op; ACT accumulates
    # samples 5-7. AG shuffles ride the tensor queue: each blocks exactly
    # the matmuls that need it.
    nc.sync.dma_start(iod[:], iod_d[:])
    XP = ctx.enter_context(tc.tile_pool(name="xp", bufs=3))
    with nc.allow_low_precision(reason="f32 sums of uint8 are exact"):
        for t in range(NT):
            np_ = 128 if t < TFULL else PPART
            xt = XP.tile([128, BPC * 1024], dt.uint8, tag="x", name=f"x{t}")
            if t < TFULL:
                nc.sync.dma_start(
                    xt[:], xf_d[:, t * BPC * 1024:(t + 1) * BPC * 1024])
            else:
                nc.sync.dma_start(xt[:np_, :], xp_d[:])
            xv = xt[:np_, :].rearrange("p (b x) -> p b x", x=1024)
            nc.vector.tensor_reduce(
                spv[:np_, t, 0:4], xv[:, 0:4, :], axis=AX.X, op=OP.add
            )
            for b in range(4, BPC):
                nc.scalar.activation(
                    acts[:np_, :],
                    xt[:np_, b * 1024:(b + 1) * 1024],
                    mybir.ActivationFunctionType.Copy,
                    accum_out=spv[:np_, t, b:b + 1],
                )
            # grouped AllGather once the last tile of a group is pooled;
            # the pt shuffle rides the scalar queue right after the group's
            # ACT accums so it blocks nothing that is needed earlier.
            for g, (t0, t1) in enumerate(AGG):
                if t == t1 - 1:
                    nc.gpsimd.dma_start(ag_in[g][:], spv[:, t0:t1, :])
                    nc.gpsimd.collective_compute(
                        "AllGather", OP.bypass, replica_groups=RG,
                        ins=[ag_in[g].opt()], outs=[ag_out[g].opt()],
                    )
                    nc.gpsimd.dma_start(
                        pt[:].rearrange("p (t c b) -> p t c b", t=NT, c=NCORES)[:, t0:t1],
                        ag_out[g][:].rearrange(
                            "c p (t b) -> p t c b", t=t1 - t0),
                    )

    # ---------------- matmul (chunk-major) + per-chunk postproc ------------
    ptv = pt[:].rearrange("p (t s) -> p t s", t=NT)
    a2v = a2a_sb[:, :QN * 6].rearrange("b (q c) -> b q c", c=6)
    sc_all = a2a_sb[:, QN * 6:]    # [B, 38] score row

    WP = ctx.enter_context(tc.tile_pool(name="wp", bufs=60))
    WPB = ctx.enter_context(tc.tile_pool(name="wpb", bufs=10))
    YPS = ctx.enter_context(tc.tile_pool(name="yps", bufs=7, space="PSUM"))

    def postproc(g, psy):
        nq = CHQ[g]
        cols = nq * NCHAN
        psv = psy[:, :cols].rearrange("b (q c) -> b q c", c=NCHAN)
        q0 = 6 * g
        sg = scores[:, g * 6: g * 6 + nq]
        # boxes + score copies ride the (otherwise idle) ACT engine
        nc.scalar.activation(
            a2v[:, q0:q0 + nq, 0:4], psv[:, :, 0:4],
            mybir.ActivationFunctionType.Copy,
        )
        nc.vector.tensor_reduce(sg, psv[:, :, 4:NCHAN], axis=AX.X, op=OP.max)
        eqv = eq[:, :nq * NCL].rearrange("b (q c) -> b q c", c=NCL)
        nc.vector.tensor_tensor(
            eqv, psv[:, :, 4:NCHAN],
            sg.unsqueeze(-1).broadcast_to((B, nq, NCL)),
            op=OP.is_ge,
        )
        amv = am[:, :nq * NCL].rearrange("b (q c) -> b q c", c=NCL)
        nc.vector.tensor_tensor(
            amv, eqv, iod[:, :].unsqueeze(1).broadcast_to((B, nq, NCL)),
            op=OP.mult,
        )
        nc.vector.tensor_reduce(argt[:, :nq], amv, axis=AX.X, op=OP.max)
        nc.vector.tensor_scalar(
            a2v[:, q0:q0 + nq, 5], argt[:, :nq], -1.0, float(NCL - 1),
            op0=OP.mult, op1=OP.add,
        )
        nc.scalar.activation(
            sc_all[:, g * 6: g * 6 + nq], sg,
            mybir.ActivationFunctionType.Copy,
        )
        nc.vector.tensor_copy(a2v[:, q0:q0 + nq, 4], sg)

    # gpsimd gather library: loading it reflashes the Q7 complex and keeps
    # the CC cores unavailable for ~70us, so it must come AFTER the
    # AllGather ladder; its drain overlaps the matmul phase and finishes
    # long before the tail's ap_gathers.
    nc.gpsimd.load_library(library_config.ap_gather)

    def w_src(g, t):
        if g < 6:
            return wA_d[g, t] if t < TFULL else wAp_d[g]
        return wB_d[t] if t < TFULL else wBp_d[:]

    psys = [YPS.tile([B, COLS[0]], F32, tag="psy", name=f"psy{g}")
            for g in range(NCHUNKS)]

    def mm(g, t, stop):
        cols = COLS[g]
        np_ = 128 if t < TFULL else PPART
        if g < 6:
            wt = WP.tile([128, COLS[0]], MM_DTYPE, tag="wt", name=f"wt{g}_{t}")
        else:
            wt = WPB.tile([128, COLS[6]], MM_DTYPE, tag="wtb", name=f"wt{g}_{t}")
        nc.sync.dma_start(wt[:np_, :cols], w_src(g, t))
        nc.tensor.matmul(
            psys[g][:, :cols], ptv[:np_, t, :], wt[:np_, :cols],
            start=(t == 0), stop=stop,
        )

    # t-major: PE runs gapless behind the W stream, gated per t-group on
    # the (strictly serialized) AllGathers; postproc pipelines per chunk
    # after its t9 matmul.
    for t in range(NT - 1):
        for g in range(NCHUNKS):
            mm(g, t, stop=False)
    for g in range(NCHUNKS):
        mm(g, NT - 1, stop=True)
        postproc(g, psys[g])
        if g < 6:
            nc.scalar.dma_start(
                a2a_in[:, 36 * g: 36 * (g + 1)],
                a2a_sb[:, 36 * g: 36 * (g + 1)])
        else:
            nc.scalar.dma_start(a2a_in[:, 216:], a2a_sb[:, 216:])

    nc.gpsimd.collective_compute(
        "AllToAll", OP.bypass, replica_groups=RG,
        ins=[a2a_in.opt()], outs=[a2a_out.opt()],
    )
    # feat[16b] rows: sample b's [304, 6] = concat of 8 cores' blocks
    nc.gpsimd.dma_start(
        swk[:].rearrange("b (c q) -> b c q", c=NCORES),
        a2a_out[:, :, QN * 6:].rearrange("c b q -> b c q"),
    )
    nc.gpsimd.dma_start(
        feat[:].rearrange("(b s) (c q x) -> b s c q x",
                          b=BPC, c=NCORES, x=6)[:, 0],
        a2a_out[:, :, :QN * 6].rearrange("c b (q x) -> b c q x", x=6),
    )
    nc.vector.memset(swk[:, NQ:NQP], NEG)  # padded queries never win

    # ---------------- top-150 tail -----------------------------------------
    # indices from early rounds are wrapped + gathered while later rounds
    # still run on DVE
    def wrap_and_gather(h):
        i0, i1 = GRAN[h]
        ic = min(i1, NROUND * 8)
        nc.vector.tensor_copy(ti16[:, i0:ic], ti[:, i0:ic])
        nc.scalar.dma_start(tsc[:, i0:i1], ti16[:, i0:i1])
        for b in range(BPC):
            eng = nc.sync if b % 2 == 0 else nc.scalar
            eng.dma_start(
                wraps[h][16 * b: 16 * b + 16, :],
                tsc[b, i0:i1].rearrange("(f p) -> p f", p=16),
            )
        nc.gpsimd.ap_gather(
            gout[:].rearrange("p (i c) -> p i c", c=6)[:, i0:i1],
            feat[:].rearrange("p (q c) -> p q c", c=6),
            wraps[h][:],
            channels=128,
            num_elems=NQP,
            d=6,
            num_idxs=i1 - i0,
        )
        o1 = min(i1, TOPK)
        if i0 < TOPK:
            nc.sync.dma_start(
                out_d[:].rearrange("b k c -> b (k c)")[:, i0 * 6: o1 * 6],
                gout[:].rearrange("(b s) x -> b s x", b=BPC)[
                    :, 0, i0 * 6: o1 * 6],
            )

    for r in range(NROUND):
        nc.vector.max(tv[:, 8 * r: 8 * r + 8], swk[:, :])
        nc.vector.max_index(ti[:, 8 * r: 8 * r + 8], tv[:, 8 * r: 8 * r + 8],
                            swk[:, :])
        if r < NROUND - 1:
            nc.vector.match_replace(
                swk[:, :], tv[:, 8 * r: 8 * r + 8], swk[:, :], NEG
            )
        if r == 5:
            wrap_and_gather(0)
        elif r == 11:
            wrap_and_gather(1)
        elif r == 17:
            wrap_and_gather(2)
    wrap_and_gather(3)




def _make_consts():
    return np.broadcast_to(
        (np.float32(NCL - 1) - np.arange(NCL, dtype=np.float32))[None, :],
        (B, NCL),
    ).copy()


_NC_CACHE = {}


def _get_nc():
    if "nc" not in _NC_CACHE:
        _NC_CACHE["nc"] = build_program()
    return _NC_CACHE["nc"]


def pack_x(xs: np.ndarray) -> tuple[np.ndarray, np.ndarray]:
    """[BPC, 3, 640, 640] int32 -> unpadded cell-major uint8 tiles.

    Cell k = c_rgb*400 + i*20 + j (matching W's row layout after the
    BGR->RGB flip); tile t<9: cell 128t+p at partition p, free b*1024+pix;
    partial tile: cells 1152..1199 on partitions 0..47.
    """
    xs8 = xs.astype(np.uint8).reshape(BPC, CHN, 20, 32, 20, 32)
    xs8 = xs8[:, ::-1]  # BGR -> RGB
    cells = xs8.transpose(0, 1, 2, 4, 3, 5).reshape(BPC, KDIM, 1024)
    full = cells[:, :128 * TFULL].reshape(BPC, TFULL, 128, 1024)
    xf = np.ascontiguousarray(full.transpose(2, 1, 0, 3)).reshape(
        128, TFULL * BPC * 1024)
    xp = np.ascontiguousarray(
        cells[:, 128 * TFULL:].transpose(1, 0, 2)).reshape(PPART, BPC * 1024)
    return xf, xp


def pack_w(W: np.ndarray):
    """[1200, 25200] -> per-core chunk-major granule tensors (scale folded).

    Returns (wA [8,6,9,128,504], wAp [8,6,48,504], wB [8,9,128,168],
    wBp [8,48,168]).
    """
    Wp = np.zeros((KDIM, NQP * NCHAN), np.float32)
    Wp[:, : NQ * NCHAN] = (W.astype(np.float64) * SCALE).astype(np.float32)
    wA = np.zeros((NCORES, 6, TFULL, 128, COLS[0]), np.float32)
    wAp = np.zeros((NCORES, 6, PPART, COLS[0]), np.float32)
    wB = np.zeros((NCORES, TFULL, 128, COLS[6]), np.float32)
    wBp = np.zeros((NCORES, PPART, COLS[6]), np.float32)
    for c in range(NCORES):
        s = Wp[:, c * QN * NCHAN: (c + 1) * QN * NCHAN]
        q0 = 0
        for g in range(NCHUNKS):
            cols = COLS[g]
            blk = s[:, q0: q0 + cols]
            if g < 6:
                wA[c, g] = blk[:128 * TFULL].reshape(TFULL, 128, cols)
                wAp[c, g] = blk[128 * TFULL:]
            else:
                wB[c] = blk[:128 * TFULL].reshape(TFULL, 128, cols)
                wBp[c] = blk[128 * TFULL:]
            q0 += cols
    return wA, wAp, wB, wBp


def make_in_maps(x: np.ndarray, W: np.ndarray) -> list[dict]:
    iod = _make_consts()
    wA, wAp, wB, wBp = pack_w(W)
    in_maps = []
    for c in range(NCORES):
        xf, xp = pack_x(x[c * BPC: (c + 1) * BPC])
        in_maps.append(
            {
                "xf": xf,
                "xp": xp,
                "wA": wA[c],
                "wAp": wAp[c],
                "wB": wB[c],
                "wBp": wBp[c],
                "iod": iod,
            }
        )
    return in_maps


def kernel(x: np.ndarray, W: np.ndarray) -> np.ndarray:
    x = np.ascontiguousarray(np.asarray(x), dtype=np.int32)
    W = np.ascontiguousarray(np.asarray(W), dtype=np.float32)
    assert x.shape == (B, CHN, HIMG, WIMG) and W.shape == (KDIM, NOUT)

    nc = _get_nc()
    in_maps = make_in_maps(x, W)
    res = run_bass_kernel_spmd(nc, in_maps, core_ids=list(range(NCORES)))
    out = np.concatenate([res.results[c]["out"] for c in range(NCORES)], axis=0)
    return out.astype(np.float32)


if __name__ == "__main__":
    xs = np.random.randint(0, 256, (B, CHN, HIMG, WIMG)).astype(np.int32)
    Ws = (np.random.randn(KDIM, NOUT) * 0.02).astype(np.float32)
    o = kernel(xs, Ws)
    print("kernel output:", o.shape, o.dtype)
